# revision 12
# baseline (speedup 1.0000x reference)
"""Trainium2 Bass kernel for the batched convex-MPC QP (nn_Cvx_Nets).

Strategy (pure data parallel, 8 cores x 128 samples):
  * Host precomputes the tiny shared MPC matrices (A_hat, B_hat, Q_hat, ...)
    from A_dyn/B_dyn/Q_sqrt/R_sqrt in numpy (O(1) work).
  * Each core solves 128 independent QPs with a fixed-iteration primal-dual
    interior point method, one sample per SBUF partition, all per-sample
    vectors along the free dimension.
  * The 91x91 KKT solve of the reference collapses: w=[z;e] with the e-block
    of the Schur matrix diagonal and a rank-1 equality constraint, so each
    IPM step needs only a per-sample 10x10 solve, done as batched
    Gauss-Jordan elimination with broadcast access patterns on the vector
    engine.  The few per-sample matvecs with shared small matrices map to
    TensorE matmuls (samples on the output partition dim).
  * The IPM is fully converged long before the reference's 20 iterations;
    we run ITERS (validated: rel err ~2e-5 at 10 iters vs the reference).
  * s,lam share one [128,360] tile (sl) and -ds,-dlam share another (SDD) so
    the step-length search and the state update are single wide DVE ops.
    The independent cc-chain runs on GpSimd+Scalar engines in parallel with
    the DVE residual chain.
"""

import numpy as np
from contextlib import ExitStack

import concourse.bacc as bacc
import concourse.tile as tile
import concourse.mybir as mybir
from concourse.bass_utils import run_bass_kernel_spmd

f32 = mybir.dt.float32
Alu = mybir.AluOpType
Act = mybir.ActivationFunctionType

N_CORES = 8
B = 1024
BPC = B // N_CORES          # 128 samples per core = partition dim
NZ, NE, NW = 10, 80, 90
MI = 180
SIGMA, REG = 0.1, 1e-8
N = 10                      # horizon
NI = 4                      # state dim
ITERS = 9


# ---------------------------------------------------------------- host prep
def _host_constants(Q_sqrt, R_sqrt, A, B_dyn):
    fnp = np.float32
    Apows = [A]
    for _ in range(1, N):
        Apows.append((Apows[-1] @ A).astype(fnp))
    A_hat = np.concatenate(Apows, axis=0)                       # (40,4)
    T = [B_dyn]
    for _ in range(1, N):
        T.append((A @ T[-1]).astype(fnp))
    T = np.stack(T)                                             # (N,4,1)
    i = np.arange(N)
    diff = i[:, None] - i[None, :]
    blocks = np.where((diff >= 0)[:, :, None, None],
                      T[np.clip(diff, 0, N - 1)], 0.0).astype(fnp)
    Bh = blocks.transpose(0, 2, 1, 3).reshape(N * NI, N).astype(fnp)  # (40,10)
    Q = (Q_sqrt @ Q_sqrt.T).astype(fnp)
    R = (R_sqrt @ R_sqrt.T).astype(fnp)
    Q_diag = np.kron(np.eye(N, dtype=fnp), Q)                   # (40,40)
    R_diag = np.kron(np.eye(N, dtype=fnp), R)                   # (10,10)
    Q_hat = (Bh.T @ (Q_diag @ Bh) + R_diag).astype(fnp)         # (10,10)
    Qhat2 = (2.0 * Q_hat).astype(fnp)

    C1 = A_hat.T.astype(fnp)                                    # (4,40)
    C2 = (2.0 * (A_hat.T @ (Q_diag @ Bh))).astype(fnp)          # (4,10)
    W4 = (A_hat.T @ Q_diag @ A_hat).astype(fnp)                 # (4,4)

    R1 = np.concatenate([Bh.T, Qhat2], axis=1).astype(fnp)      # (10,50)

    P40 = np.einsum('ki,kj->kij', Bh, Bh).reshape(40, 100).astype(fnp)
    RBug = np.zeros((80, 20), fnp)                              # [u_vec|gdiff]@.
    RBug[0:40, 0:10] = Bh
    RBug[40:80, 10:20] = Bh

    QFrow = (Qhat2 + REG * np.eye(NZ, dtype=fnp)).reshape(-1).astype(fnp)
    QF = np.ascontiguousarray(np.broadcast_to(QFrow, (BPC, 100))).astype(fnp)

    E0P = np.zeros((BPC, 20), fnp)
    E0P[:, 1] = 1.0                                             # e0 rhs col, row 0

    ID = np.eye(BPC, dtype=fnp)
    return dict(R1=R1, P40=P40, RBug=RBug, QF=QF, E0P=E0P, ID=ID,
                C1=C1, C2=C2, W4=W4, Q=Q)


# ------------------------------------------------------------- device build
def build_program(iters=ITERS):
    nc = bacc.Bacc("TRN2", target_bir_lowering=False, debug=False)
    P = BPC

    h2in = nc.dram_tensor("h2in", [P, NE], f32, kind="ExternalInput").ap()
    pin = nc.dram_tensor("pin", [P, NZ], f32, kind="ExternalInput").ap()
    bcin = nc.dram_tensor("bcin", [P, 1], f32, kind="ExternalInput").ap()
    u0in = nc.dram_tensor("u0in", [P, 1], f32, kind="ExternalInput").ap()
    r1d = nc.dram_tensor("r1", [NZ, 50], f32, kind="ExternalInput").ap()
    p40d = nc.dram_tensor("p40", [40, 100], f32, kind="ExternalInput").ap()
    rugd = nc.dram_tensor("rug", [80, 20], f32, kind="ExternalInput").ap()
    qfd = nc.dram_tensor("qf", [P, 100], f32, kind="ExternalInput").ap()
    e0d = nc.dram_tensor("e0p", [P, 20], f32, kind="ExternalInput").ap()
    idd = nc.dram_tensor("ident", [P, P], f32, kind="ExternalInput").ap()
    cost_out = nc.dram_tensor("cost", [P, 1], f32, kind="ExternalOutput").ap()
    u0_out = nc.dram_tensor("u0o", [P, 1], f32, kind="ExternalOutput").ap()

    with tile.TileContext(nc) as tc, ExitStack() as ctx:
        sb = ctx.enter_context(tc.tile_pool(name="sb", bufs=1))
        ps = ctx.enter_context(tc.tile_pool(name="ps", bufs=1, space="PSUM"))

        def st(shape, tag):
            return sb.tile(list(shape), f32, tag=tag, name=tag)

        # constants
        R1 = st((P, 50), "R1")
        P40 = st((P, 100), "P40"); RUG = st((P, 20), "RUG")
        QF = st((P, 100), "QF"); E0P = st((P, 20), "E0P"); ID = st((P, P), "ID")
        # state
        beq = st((P, 1), "beq")
        w = st((P, NW), "w")
        sl = st((P, 2 * MI), "sl")          # [s | lam]
        nuneg = st((P, 1), "nuneg")
        p = st((P, NZ), "p"); h2 = st((P, NE), "h2"); bc = st((P, 1), "bc")
        # per-iteration tensors
        rp = st((P, MI), "rp"); rs = st((P, MI), "rs"); d = st((P, MI), "d")
        mus = st((P, MI), "mus"); lt = st((P, MI), "lt")
        t180 = st((P, MI), "t180"); v1 = st((P, MI), "v1")
        me1 = st((P, NE), "me1"); me = st((P, NE), "me"); rme = st((P, NE), "rme")
        t_cc1 = st((P, NE), "t_cc1"); t_cc2 = st((P, NE), "t_cc2")
        cct = st((P, 40), "cct")            # cc40 (gpsimd-owned)
        t_rpa = st((P, NE), "t_rpa"); t_rpb = st((P, NE), "t_rpb")
        t_lt23 = st((P, NE), "t_lt23"); rw_e = st((P, NE), "rw_e")
        t_ga = st((P, NE), "t_ga"); t_gb = st((P, NE), "t_gb")
        vtcat = st((P, 80), "vtcat")        # [u_vec | gdiff]
        t_lt1d = st((P, NZ), "t_lt1d"); t_zhp = st((P, NZ), "t_zhp")
        t_mbg = st((P, NZ), "t_mbg"); dg = st((P, NZ), "dg")
        rn = st((P, 1), "rn")
        T = st((P, 120), "T"); recips = st((P, NZ), "recips")
        prod = st((P, 112), "prod"); X = st((P, 20), "X")
        den = st((P, 1), "den"); rden = st((P, 1), "rden")
        num2 = st((P, 1), "num2"); dnu_neg = st((P, 1), "dnu_neg")
        dw = st((P, NW), "dw"); de80 = st((P, NE), "de80")
        t80ab = st((P, NE), "t80ab"); u80 = st((P, NE), "u80")
        SDD = st((P, 2 * MI), "SDD")        # [-ds | -dlam]
        sm = st((P, 2 * MI), "sm"); rm = st((P, 2 * MI), "rm")
        ratio = st((P, 2 * MI), "ratio")
        junkA = st((P, MI), "junkA"); junk10 = st((P, NZ), "junk10")
        junk80 = st((P, NE), "junk80")
        slam = st((P, 1), "slam")
        a1t = st((P, 1), "a1t")
        alpha = st((P, 1), "alpha"); alphan = st((P, 1), "alphan")
        bt_ = st((P, 1), "bt_"); ct_ = st((P, 1), "ct_")
        costsb = st((P, 1), "costsb")
        tT = st((P, P), "tT"); tTc = st((P, P), "tTc")

        tps = ps.tile([P, P], f32, tag="tps", name="tps")
        tpsC = ps.tile([P, P], f32, tag="tpsC", name="tpsC")
        mmA = ps.tile([P, 64], f32, tag="mmA", name="mmA")
        mmU = ps.tile([P, 32], f32, tag="mmU", name="mmU")
        mmC = ps.tile([P, 128], f32, tag="mmC", name="mmC")

        V = nc.vector
        SC = nc.scalar
        TE = nc.tensor
        GP = nc.gpsimd

        # views
        s_ = sl[:, 0:MI]
        lam_ = sl[:, MI:2 * MI]
        SD_ = SDD[:, 0:MI]
        dlN_ = SDD[:, MI:2 * MI]
        Tv = T[:].rearrange("p (r c) -> p r c", c=12)
        Tdiag = T[:, 0:120:13]
        Xv = X[:].rearrange("p (r c) -> p r c", c=2)
        E0Pv = E0P[:].rearrange("p (r c) -> p r c", c=2)
        mmC_S = mmC[:, 0:100].rearrange("p (r c) -> p r c", c=10)
        QFv = QF[:].rearrange("p (r c) -> p r c", c=10)
        recv = recips[:].rearrange("p (r c) -> p r c", c=1)

        # ------------------------------------------------ load + init
        nc.sync.dma_start(h2[:], h2in)
        nc.sync.dma_start(p[:], pin)
        nc.sync.dma_start(bc[:], bcin)
        nc.sync.dma_start(beq[:], u0in)
        nc.sync.dma_start(R1[:NZ, :], r1d)
        nc.sync.dma_start(P40[:40, :], p40d)
        nc.sync.dma_start(RUG[:80, :], rugd)
        nc.sync.dma_start(QF[:], qfd)
        nc.sync.dma_start(E0P[:], e0d)
        nc.sync.dma_start(ID[:], idd)

        GP.memset(sl[:, 0:20], 1.0)
        V.tensor_scalar(sl[:, 20:100], h2[:], 1.0, None, Alu.max)
        GP.memset(sl[:, 100:180], 1.0)
        GP.memset(lam_, 1.0)
        GP.memset(w[:], 0.0)
        GP.memset(nuneg[:], 0.0)

        # ------------------------------------------------ IPM iterations
        for it in range(iters):
            # z transpose + MM1: Bhz | z@2Q_hat
            TE.transpose(tps[:NZ, :], w[:, :NZ], ID[:])
            SC.copy(tT[:NZ, :], tps[:NZ, :])
            TE.matmul(mmA[:, :50], tT[:NZ, :], R1[:NZ, :])

            # mu, rs, d  (independent of MM1)
            V.scalar_tensor_tensor(junkA[:], s_, 1.0, lam_,
                                   Alu.mult, Alu.mult, accum_out=slam[:])
            V.reciprocal_approx_fast(rs[:], s_)
            V.tensor_tensor(d[:], lam_, rs[:], Alu.mult)
            V.tensor_scalar(mus[:], rs[:], slam[:], SIGMA / MI,
                            Alu.mult, Alu.mult)
            # cc-chain: cc40 = d2 - d2*(d2/me) (+halves); me1/t_cc1 on GpSimd
            GP.tensor_tensor(me1[:], d[:, 20:100], d[:, 100:180], Alu.add)
            SC.activation(me[:], me1[:], Act.Copy, bias=2.0)
            V.reciprocal_approx_fast(rme[:], me[:])
            GP.tensor_tensor(t_cc1[:], d[:, 20:100], rme[:], Alu.mult)
            V.tensor_tensor(t_cc2[:], t_cc1[:], d[:, 20:100], Alu.mult)
            V.tensor_tensor(t_cc2[:], d[:, 20:100], t_cc2[:], Alu.subtract)
            V.tensor_tensor(cct[:], t_cc2[:, :40], t_cc2[:, 40:], Alu.add)
            # cc transpose + MM-cc: Sg = cc40 @ P40   (early, overlaps)
            TE.transpose(tpsC[:40, :], cct[:], ID[:])
            SC.copy(tTc[:40, :], tpsC[:40, :])
            TE.matmul(mmC[:, :100], tTc[:40, :], P40[:40, :])

            # r_prim (needs Bhz)
            V.scalar_tensor_tensor(rp[:, :10], sl[:, :10], -0.5, w[:, :10],
                                   Alu.add, Alu.add)
            V.scalar_tensor_tensor(rp[:, 10:20], sl[:, 10:20], -0.5, w[:, :10],
                                   Alu.add, Alu.subtract)
            GP.tensor_tensor(t_rpa[:], sl[:, 20:100], h2[:], Alu.subtract)
            GP.tensor_tensor(t_rpb[:], t_rpa[:], w[:, 10:90], Alu.subtract)
            V.tensor_tensor(rp[:, 20:60], t_rpb[:, :40], mmA[:, :40], Alu.add)
            V.tensor_tensor(rp[:, 60:100], t_rpb[:, 40:], mmA[:, :40],
                            Alu.subtract)
            V.tensor_tensor(rp[:, 100:180], sl[:, 100:180], w[:, 10:90],
                            Alu.subtract)

            # lt = lam + tmp = d*rp + mus;  v1 = lam - mus (for dlam, gpsimd)
            V.tensor_tensor(t180[:], d[:], rp[:], Alu.mult)
            V.tensor_tensor(lt[:], t180[:], mus[:], Alu.add)
            GP.tensor_tensor(v1[:], lam_, mus[:], Alu.subtract)

            # gdiff chain (feeds MM-ug): rw_e, g
            V.tensor_tensor(t_lt23[:], lt[:, 20:100], lt[:, 100:180], Alu.add)
            V.scalar_tensor_tensor(rw_e[:], w[:, 10:90], -2.0, t_lt23[:],
                                   Alu.mult, Alu.add)
            V.tensor_tensor(t_gb[:], t_cc1[:], rw_e[:], Alu.mult)
            V.tensor_tensor(vtcat[:, 40:80], t_gb[:, :40], t_gb[:, 40:],
                            Alu.subtract)
            V.tensor_tensor(vtcat[:, :40], lt[:, 20:60], lt[:, 60:100],
                            Alu.subtract)

            # MM-ug: [u_vec|gdiff] @ blockdiag(Bh,Bh) -> MB_u | MB_g
            TE.transpose(tps[:80, :], vtcat[:], ID[:])
            V.tensor_copy(tT[:80, :], tps[:80, :])
            TE.matmul(mmU[:, :20], tT[:80, :], RUG[:80, :])

            # rw_z pieces, rn (overlap MM-ug)
            GP.tensor_tensor(t_lt1d[:], lt[:, :10], lt[:, 10:20], Alu.subtract)
            V.tensor_tensor(t_zhp[:], mmA[:, 40:50], p[:], Alu.add)
            V.tensor_tensor(t_zhp[:], t_zhp[:], t_lt1d[:], Alu.add)
            GP.tensor_tensor(rn[:], beq[:], w[:, 0:1], Alu.subtract)
            GP.tensor_tensor(dg[:], d[:, :10], d[:, 10:20], Alu.add)

            # tableau assembly
            V.tensor_tensor(Tv[:, :, 0:10], mmC_S, QFv, Alu.add)
            SC.copy(Tv[:, :, 10:12], E0Pv)
            V.tensor_tensor(Tdiag, Tdiag, dg[:], Alu.add)
            V.tensor_tensor(t_mbg[:], mmU[:, 10:20], t_zhp[:], Alu.subtract)
            V.tensor_tensor(Tv[:, :, 10:11],
                            t_mbg[:].rearrange("p (r c) -> p r c", c=1),
                            mmU[:, 0:10].rearrange("p (r c) -> p r c", c=1),
                            Alu.subtract)
            V.tensor_tensor(T[:, 10:11], T[:, 10:11], nuneg[:], Alu.add)

            # batched 10x10 Gauss-Jordan, 2 rhs (rt | e0)
            for k in range(10):
                V.reciprocal_approx_fast(recips[:, k:k + 1], T[:, 13 * k:13 * k + 1])
                mcols = 11 - k
                pv = prod[:, :10 * mcols].rearrange("p (r c) -> p r c", c=mcols)
                V.scalar_tensor_tensor(
                    pv,
                    Tv[:, :, k:k + 1].broadcast_to((P, 10, mcols)),
                    recips[:, k:k + 1],
                    Tv[:, k:k + 1, k + 1:].broadcast_to((P, 10, mcols)),
                    Alu.mult, Alu.mult)
                if k < 9:
                    V.tensor_tensor(Tv[:, k + 1:, k + 1:],
                                    Tv[:, k + 1:, k + 1:], pv[:, k + 1:, :],
                                    Alu.subtract)
                if k > 0:
                    V.tensor_tensor(Tv[:, :k, k + 1:],
                                    Tv[:, :k, k + 1:], pv[:, :k, :],
                                    Alu.subtract)
            V.tensor_tensor(Xv, Tv[:, :, 10:12],
                            recv.broadcast_to((P, 10, 2)), Alu.mult)

            # dnu (negated), dz
            V.tensor_scalar(den[:], X[:, 1:2], REG, None, Alu.add)
            V.reciprocal_approx_fast(rden[:], den[:])
            V.tensor_tensor(num2[:], rn[:], X[:, 0:1], Alu.subtract)
            V.tensor_tensor(dnu_neg[:], num2[:], rden[:], Alu.mult)
            V.scalar_tensor_tensor(
                dw[:, :10].rearrange("p (r c) -> p r c", c=1),
                Xv[:, :, 1:2], dnu_neg[:], Xv[:, :, 0:1], Alu.mult, Alu.add)

            # MM3: Bhdz
            TE.transpose(tps[:NZ, :], dw[:, :NZ], ID[:])
            V.tensor_copy(tT[:NZ, :], tps[:NZ, :])
            TE.matmul(mmA[:, :40], tT[:NZ, :], R1[:NZ, :40])

            # SD parts that need only dz (overlap MM3)
            V.tensor_tensor(SDD[:, :10], dw[:, :10], rp[:, :10], Alu.add)
            V.tensor_tensor(SDD[:, 10:20], rp[:, 10:20], dw[:, :10],
                            Alu.subtract)

            # de = (rw_e + d2*[Bhdz;-Bhdz]) / me
            V.tensor_tensor(
                t80ab[:].rearrange("p (r c) -> p r c", c=40),
                d[:, 20:100].rearrange("p (r c) -> p r c", c=40),
                mmA[:, :40].unsqueeze(1).broadcast_to((P, 2, 40)),
                Alu.mult)
            V.tensor_tensor(de80[:, :40], rw_e[:, :40], t80ab[:, :40], Alu.add)
            V.tensor_tensor(de80[:, 40:], rw_e[:, 40:], t80ab[:, 40:],
                            Alu.subtract)
            V.tensor_tensor(dw[:, 10:90], de80[:], rme[:], Alu.mult)

            # SD = -(ds) = G dw + r_prim (rest)
            V.tensor_tensor(u80[:], rp[:, 20:100], dw[:, 10:90], Alu.subtract)
            V.tensor_tensor(SDD[:, 20:60], u80[:, :40], mmA[:, :40], Alu.add)
            V.tensor_tensor(SDD[:, 60:100], u80[:, 40:], mmA[:, :40],
                            Alu.subtract)
            V.tensor_tensor(SDD[:, 100:180], rp[:, 100:180], dw[:, 10:90],
                            Alu.subtract)

            # -dlam = (lam - mus) - d*SD
            V.tensor_tensor(t180[:], d[:], SD_, Alu.mult)
            V.tensor_tensor(dlN_, v1[:], t180[:], Alu.subtract)

            # alpha = 0.99*min(1, min([s|lam] / max(SDD, eps)))
            V.tensor_scalar(sm[:], SDD[:], 1e-12, None, Alu.max)
            V.reciprocal_approx_fast(rm[:], sm[:])
            V.tensor_tensor(ratio[:], sl[:], rm[:], Alu.mult)
            V.tensor_reduce(a1t[:], ratio[:], mybir.AxisListType.X, Alu.min)
            V.tensor_scalar(alpha[:], a1t[:], 1.0, 0.99, Alu.min, Alu.mult)
            V.tensor_scalar(alphan[:], alpha[:], -1.0, None, Alu.mult)

            # updates
            V.scalar_tensor_tensor(w[:], dw[:], alpha[:], w[:],
                                   Alu.mult, Alu.add)
            V.scalar_tensor_tensor(sl[:], SDD[:], alphan[:], sl[:],
                                   Alu.mult, Alu.add)
            V.scalar_tensor_tensor(nuneg[:], dnu_neg[:], alpha[:], nuneg[:],
                                   Alu.mult, Alu.add)

        # ------------------------------------------------ final cost
        TE.transpose(tps[:NZ, :], w[:, :NZ], ID[:])
        SC.copy(tT[:NZ, :], tps[:NZ, :])
        TE.matmul(mmA[:, :50], tT[:NZ, :], R1[:NZ, :])
        V.scalar_tensor_tensor(t_zhp[:], mmA[:, 40:50], 0.5, p[:],
                               Alu.mult, Alu.add)        # u@Q_hat + p
        V.scalar_tensor_tensor(junk10[:], t_zhp[:], 1.0, w[:, :10],
                               Alu.mult, Alu.mult, accum_out=bt_[:])
        V.scalar_tensor_tensor(junk80[:], w[:, 10:], 1.0, w[:, 10:],
                               Alu.mult, Alu.mult, accum_out=ct_[:])
        V.tensor_tensor(costsb[:], bt_[:], ct_[:], Alu.add)
        V.tensor_tensor(costsb[:], costsb[:], bc[:], Alu.add)
        nc.sync.dma_start(cost_out, costsb[:])
        nc.sync.dma_start(u0_out, w[:, 0:1])

    nc.compile()
    return nc


# ------------------------------------------------------------------- driver
_CACHE = {}


def _get_program(iters=ITERS):
    if iters not in _CACHE:
        _CACHE[iters] = build_program(iters)
    return _CACHE[iters]


def make_in_maps(x, u0, Q_sqrt, R_sqrt, A_dyn, B_dyn):
    fnp = np.float32
    consts = _host_constants(np.asarray(Q_sqrt, fnp), np.asarray(R_sqrt, fnp),
                             np.asarray(A_dyn, fnp), np.asarray(B_dyn, fnp))
    us = np.ascontiguousarray(np.asarray(u0, fnp).reshape(N_CORES, BPC, 1))
    shared = {
        "r1": consts["R1"], "p40": consts["P40"],
        "rug": consts["RBug"], "qf": consts["QF"], "e0p": consts["E0P"],
        "ident": consts["ID"],
    }
    xf = np.asarray(x, fnp)
    A_x0 = (xf @ consts["C1"]).astype(fnp)                      # (B,40)
    h2f = np.concatenate([4.0 - A_x0, 4.0 + A_x0], axis=1).astype(fnp)
    pf = (xf @ consts["C2"]).astype(fnp)                        # (B,10)
    bcf = (np.sum((xf @ consts["W4"]) * xf, axis=1)
           + np.sum((xf @ consts["Q"]) * xf, axis=1)).astype(fnp)[:, None]
    h2s = h2f.reshape(N_CORES, BPC, NE)
    ps_ = pf.reshape(N_CORES, BPC, NZ)
    bcs = bcf.reshape(N_CORES, BPC, 1)
    return [dict(shared, h2in=h2s[c], pin=ps_[c], bcin=bcs[c], u0in=us[c])
            for c in range(N_CORES)]


def kernel(x, u0, Q_sqrt, R_sqrt, A_dyn, B_dyn):
    nc = _get_program()
    in_maps = make_in_maps(x, u0, Q_sqrt, R_sqrt, A_dyn, B_dyn)
    res = run_bass_kernel_spmd(nc, in_maps, list(range(N_CORES)))
    cost = np.concatenate([res.results[c]["cost"] for c in range(N_CORES)],
                          axis=0).astype(np.float32)
    u0o = np.concatenate([res.results[c]["u0o"] for c in range(N_CORES)],
                         axis=0).reshape(B).astype(np.float32)
    return cost, u0o


# revision 14
# speedup vs baseline: 1.0485x; 1.0485x over previous
"""Trainium2 Bass kernel for the batched convex-MPC QP (nn_Cvx_Nets).

Strategy (pure data parallel, 8 cores x 128 samples):
  * Host precomputes the tiny shared MPC matrices (A_hat, B_hat, Q_hat, ...)
    from A_dyn/B_dyn/Q_sqrt/R_sqrt in numpy (O(1) work).
  * Each core solves 128 independent QPs with a fixed-iteration primal-dual
    interior point method, one sample per SBUF partition, all per-sample
    vectors along the free dimension.
  * The 91x91 KKT solve of the reference collapses: w=[z;e] with the e-block
    of the Schur matrix diagonal and a rank-1 equality constraint, so each
    IPM step needs only a per-sample 10x10 solve, done as batched
    Gauss-Jordan elimination with broadcast access patterns on the vector
    engine.  The few per-sample matvecs with shared small matrices map to
    TensorE matmuls (samples on the output partition dim).
  * The IPM is fully converged long before the reference's 20 iterations;
    we run ITERS (validated: rel err ~2e-5 at 10 iters vs the reference).
  * s,lam share one [128,360] tile (sl) and -ds,-dlam share another (SDD) so
    the step-length search and the state update are single wide DVE ops.
    The independent cc-chain runs on GpSimd+Scalar engines in parallel with
    the DVE residual chain.
"""

import numpy as np
from contextlib import ExitStack

import concourse.bacc as bacc
import concourse.tile as tile
import concourse.mybir as mybir
from concourse.bass_utils import run_bass_kernel_spmd

f32 = mybir.dt.float32
Alu = mybir.AluOpType
Act = mybir.ActivationFunctionType

N_CORES = 8
B = 1024
BPC = B // N_CORES          # 128 samples per core = partition dim
NZ, NE, NW = 10, 80, 90
MI = 180
SIGMA, REG = 0.1, 1e-8
N = 10                      # horizon
NI = 4                      # state dim
ITERS = 9


# ---------------------------------------------------------------- host prep
def _host_constants(Q_sqrt, R_sqrt, A, B_dyn):
    fnp = np.float32
    Apows = [A]
    for _ in range(1, N):
        Apows.append((Apows[-1] @ A).astype(fnp))
    A_hat = np.concatenate(Apows, axis=0)                       # (40,4)
    T = [B_dyn]
    for _ in range(1, N):
        T.append((A @ T[-1]).astype(fnp))
    T = np.stack(T)                                             # (N,4,1)
    i = np.arange(N)
    diff = i[:, None] - i[None, :]
    blocks = np.where((diff >= 0)[:, :, None, None],
                      T[np.clip(diff, 0, N - 1)], 0.0).astype(fnp)
    Bh = blocks.transpose(0, 2, 1, 3).reshape(N * NI, N).astype(fnp)  # (40,10)
    Q = (Q_sqrt @ Q_sqrt.T).astype(fnp)
    R = (R_sqrt @ R_sqrt.T).astype(fnp)
    Q_diag = np.kron(np.eye(N, dtype=fnp), Q)                   # (40,40)
    R_diag = np.kron(np.eye(N, dtype=fnp), R)                   # (10,10)
    Q_hat = (Bh.T @ (Q_diag @ Bh) + R_diag).astype(fnp)         # (10,10)
    Qhat2 = (2.0 * Q_hat).astype(fnp)

    C1 = A_hat.T.astype(fnp)                                    # (4,40)
    C2 = (2.0 * (A_hat.T @ (Q_diag @ Bh))).astype(fnp)          # (4,10)
    W4 = (A_hat.T @ Q_diag @ A_hat).astype(fnp)                 # (4,4)

    R1 = np.concatenate([Bh.T, Qhat2], axis=1).astype(fnp)      # (10,50)

    P40 = np.einsum('ki,kj->kij', Bh, Bh).reshape(40, 100).astype(fnp)
    RBug = np.zeros((80, 20), fnp)                              # [u_vec|gdiff]@.
    RBug[0:40, 0:10] = Bh
    RBug[40:80, 10:20] = Bh

    QFrow = (Qhat2 + REG * np.eye(NZ, dtype=fnp)).reshape(-1).astype(fnp)
    QF = np.ascontiguousarray(np.broadcast_to(QFrow, (BPC, 100))).astype(fnp)

    E0P = np.zeros((BPC, 20), fnp)
    E0P[:, 1] = 1.0                                             # e0 rhs col, row 0

    ID = np.eye(BPC, dtype=fnp)
    return dict(R1=R1, P40=P40, RBug=RBug, QF=QF, E0P=E0P, ID=ID,
                C1=C1, C2=C2, W4=W4, Q=Q)


# ------------------------------------------------------------- device build
def build_program(iters=ITERS):
    nc = bacc.Bacc("TRN2", target_bir_lowering=False, debug=False)
    P = BPC

    h2in = nc.dram_tensor("h2in", [P, NE], f32, kind="ExternalInput").ap()
    pin = nc.dram_tensor("pin", [P, NZ], f32, kind="ExternalInput").ap()
    bcin = nc.dram_tensor("bcin", [P, 1], f32, kind="ExternalInput").ap()
    u0in = nc.dram_tensor("u0in", [P, 1], f32, kind="ExternalInput").ap()
    r1d = nc.dram_tensor("r1", [NZ, 50], f32, kind="ExternalInput").ap()
    p40d = nc.dram_tensor("p40", [40, 100], f32, kind="ExternalInput").ap()
    rugd = nc.dram_tensor("rug", [80, 20], f32, kind="ExternalInput").ap()
    qfd = nc.dram_tensor("qf", [P, 100], f32, kind="ExternalInput").ap()
    e0d = nc.dram_tensor("e0p", [P, 20], f32, kind="ExternalInput").ap()
    idd = nc.dram_tensor("ident", [P, P], f32, kind="ExternalInput").ap()
    cost_out = nc.dram_tensor("cost", [P, 1], f32, kind="ExternalOutput").ap()
    u0_out = nc.dram_tensor("u0o", [P, 1], f32, kind="ExternalOutput").ap()

    with tile.TileContext(nc) as tc, ExitStack() as ctx:
        sb = ctx.enter_context(tc.tile_pool(name="sb", bufs=1))
        ps = ctx.enter_context(tc.tile_pool(name="ps", bufs=1, space="PSUM"))

        def st(shape, tag):
            return sb.tile(list(shape), f32, tag=tag, name=tag)

        # constants
        R1 = st((P, 50), "R1")
        P40 = st((P, 100), "P40"); RUG = st((P, 20), "RUG")
        QF = st((P, 100), "QF"); E0P = st((P, 20), "E0P"); ID = st((P, P), "ID")
        # state
        beq = st((P, 1), "beq")
        w = st((P, NW), "w")
        sl = st((P, 2 * MI), "sl")          # [s | lam]
        nuneg = st((P, 1), "nuneg")
        p = st((P, NZ), "p"); h2 = st((P, NE), "h2"); bc = st((P, 1), "bc")
        # per-iteration tensors
        rp = st((P, MI), "rp"); rs = st((P, MI), "rs"); d = st((P, MI), "d")
        mus = st((P, MI), "mus"); lt = st((P, MI), "lt")
        t180 = st((P, MI), "t180"); v1 = st((P, MI), "v1")
        me1 = st((P, NE), "me1"); me = st((P, NE), "me"); rme = st((P, NE), "rme")
        t_cc1 = st((P, NE), "t_cc1"); t_cc2 = st((P, NE), "t_cc2")
        cct = st((P, 40), "cct")            # cc40 (gpsimd-owned)
        t_rpa = st((P, NE), "t_rpa"); t_rpb = st((P, NE), "t_rpb")
        t_lt23 = st((P, NE), "t_lt23"); rw_e = st((P, NE), "rw_e")
        t_ga = st((P, NE), "t_ga"); t_gb = st((P, NE), "t_gb")
        vtcat = st((P, 80), "vtcat")        # [u_vec | gdiff]
        t_lt1d = st((P, NZ), "t_lt1d"); t_zhp = st((P, NZ), "t_zhp")
        t_mbg = st((P, NZ), "t_mbg"); dg = st((P, NZ), "dg")
        rn = st((P, 1), "rn")
        T = st((P, 120), "T"); recips = st((P, NZ), "recips")
        prod = st((P, 112), "prod"); X = st((P, 20), "X")
        den = st((P, 1), "den"); rden = st((P, 1), "rden")
        num2 = st((P, 1), "num2"); dnu_neg = st((P, 1), "dnu_neg")
        dw = st((P, NW), "dw"); de80 = st((P, NE), "de80")
        t80ab = st((P, NE), "t80ab"); u80 = st((P, NE), "u80")
        SDD = st((P, 2 * MI), "SDD")        # [-ds | -dlam]
        sm = st((P, 2 * MI), "sm"); rm = st((P, 2 * MI), "rm")
        ratio = st((P, 2 * MI), "ratio")
        junkA = st((P, MI), "junkA"); junk10 = st((P, NZ), "junk10")
        junk80 = st((P, NE), "junk80")
        slam = st((P, 1), "slam")
        a1t = st((P, 1), "a1t")
        alpha = st((P, 1), "alpha"); alphan = st((P, 1), "alphan")
        bt_ = st((P, 1), "bt_"); ct_ = st((P, 1), "ct_")
        costsb = st((P, 1), "costsb")
        tT = st((P, P), "tT"); tTc = st((P, P), "tTc")

        tps = ps.tile([P, P], f32, tag="tps", name="tps")
        tpsC = ps.tile([P, P], f32, tag="tpsC", name="tpsC")
        mmA = ps.tile([P, 64], f32, tag="mmA", name="mmA")
        mmU = ps.tile([P, 32], f32, tag="mmU", name="mmU")
        mmC = ps.tile([P, 128], f32, tag="mmC", name="mmC")

        V = nc.vector
        SC = nc.scalar
        TE = nc.tensor
        GP = nc.gpsimd

        # views
        s_ = sl[:, 0:MI]
        lam_ = sl[:, MI:2 * MI]
        SD_ = SDD[:, 0:MI]
        dlN_ = SDD[:, MI:2 * MI]
        Tv = T[:].rearrange("p (r c) -> p r c", c=12)
        Tdiag = T[:, 0:120:13]
        Xv = X[:].rearrange("p (r c) -> p r c", c=2)
        E0Pv = E0P[:].rearrange("p (r c) -> p r c", c=2)
        mmC_S = mmC[:, 0:100].rearrange("p (r c) -> p r c", c=10)
        QFv = QF[:].rearrange("p (r c) -> p r c", c=10)
        recv = recips[:].rearrange("p (r c) -> p r c", c=1)

        # ------------------------------------------------ load + init
        nc.sync.dma_start(h2[:], h2in)
        nc.sync.dma_start(p[:], pin)
        nc.sync.dma_start(bc[:], bcin)
        nc.sync.dma_start(beq[:], u0in)
        nc.sync.dma_start(R1[:NZ, :], r1d)
        nc.sync.dma_start(P40[:40, :], p40d)
        nc.sync.dma_start(RUG[:80, :], rugd)
        nc.sync.dma_start(QF[:], qfd)
        nc.sync.dma_start(E0P[:], e0d)
        nc.sync.dma_start(ID[:], idd)

        GP.memset(sl[:, 0:20], 1.0)
        V.tensor_scalar(sl[:, 20:100], h2[:], 1.0, None, Alu.max)
        GP.memset(sl[:, 100:180], 1.0)
        GP.memset(lam_, 1.0)
        GP.memset(w[:], 0.0)
        GP.memset(nuneg[:], 0.0)

        # ------------------------------------------------ IPM iterations
        for it in range(iters):
            # z transpose + MM1: Bhz | z@2Q_hat
            TE.transpose(tps[:NZ, :], w[:, :NZ], ID[:])
            SC.copy(tT[:NZ, :], tps[:NZ, :])
            TE.matmul(mmA[:, :50], tT[:NZ, :], R1[:NZ, :])

            # mu, rs, d  (independent of MM1)
            V.scalar_tensor_tensor(junkA[:], s_, 1.0, lam_,
                                   Alu.mult, Alu.mult, accum_out=slam[:])
            V.reciprocal(rs[:], s_)
            V.tensor_tensor(d[:], lam_, rs[:], Alu.mult)
            V.tensor_scalar(mus[:], rs[:], slam[:], SIGMA / MI,
                            Alu.mult, Alu.mult)
            # cc-chain: cc40 = d2 - d2*(d2/me) (+halves); me1/t_cc1 on GpSimd
            GP.tensor_tensor(me1[:], d[:, 20:100], d[:, 100:180], Alu.add)
            SC.activation(me[:], me1[:], Act.Copy, bias=2.0)
            V.reciprocal(rme[:], me[:])
            GP.tensor_tensor(t_cc1[:], d[:, 20:100], rme[:], Alu.mult)
            V.tensor_tensor(t_cc2[:], t_cc1[:], d[:, 20:100], Alu.mult)
            V.tensor_tensor(t_cc2[:], d[:, 20:100], t_cc2[:], Alu.subtract)
            V.tensor_tensor(cct[:], t_cc2[:, :40], t_cc2[:, 40:], Alu.add)
            # cc transpose + MM-cc: Sg = cc40 @ P40   (early, overlaps)
            TE.transpose(tpsC[:40, :], cct[:], ID[:])
            SC.copy(tTc[:40, :], tpsC[:40, :])
            TE.matmul(mmC[:, :100], tTc[:40, :], P40[:40, :])

            # r_prim (needs Bhz)
            V.scalar_tensor_tensor(rp[:, :10], sl[:, :10], -0.5, w[:, :10],
                                   Alu.add, Alu.add)
            V.scalar_tensor_tensor(rp[:, 10:20], sl[:, 10:20], -0.5, w[:, :10],
                                   Alu.add, Alu.subtract)
            GP.tensor_tensor(t_rpa[:], sl[:, 20:100], h2[:], Alu.subtract)
            GP.tensor_tensor(t_rpb[:], t_rpa[:], w[:, 10:90], Alu.subtract)
            V.tensor_tensor(rp[:, 20:60], t_rpb[:, :40], mmA[:, :40], Alu.add)
            V.tensor_tensor(rp[:, 60:100], t_rpb[:, 40:], mmA[:, :40],
                            Alu.subtract)
            V.tensor_tensor(rp[:, 100:180], sl[:, 100:180], w[:, 10:90],
                            Alu.subtract)

            # lt = lam + tmp = d*rp + mus;  v1 = lam - mus (for dlam, gpsimd)
            V.tensor_tensor(t180[:], d[:], rp[:], Alu.mult)
            V.tensor_tensor(lt[:], t180[:], mus[:], Alu.add)
            GP.tensor_tensor(v1[:], lam_, mus[:], Alu.subtract)

            # gdiff chain (feeds MM-ug): rw_e, g
            V.tensor_tensor(t_lt23[:], lt[:, 20:100], lt[:, 100:180], Alu.add)
            V.scalar_tensor_tensor(rw_e[:], w[:, 10:90], -2.0, t_lt23[:],
                                   Alu.mult, Alu.add)
            V.tensor_tensor(t_gb[:], t_cc1[:], rw_e[:], Alu.mult)
            V.tensor_tensor(vtcat[:, 40:80], t_gb[:, :40], t_gb[:, 40:],
                            Alu.subtract)
            V.tensor_tensor(vtcat[:, :40], lt[:, 20:60], lt[:, 60:100],
                            Alu.subtract)

            # MM-ug: [u_vec|gdiff] @ blockdiag(Bh,Bh) -> MB_u | MB_g
            TE.transpose(tps[:80, :], vtcat[:], ID[:])
            V.tensor_copy(tT[:80, :], tps[:80, :])
            TE.matmul(mmU[:, :20], tT[:80, :], RUG[:80, :])

            # rw_z pieces, rn (overlap MM-ug)
            GP.tensor_tensor(t_lt1d[:], lt[:, :10], lt[:, 10:20], Alu.subtract)
            V.tensor_tensor(t_zhp[:], mmA[:, 40:50], p[:], Alu.add)
            V.tensor_tensor(t_zhp[:], t_zhp[:], t_lt1d[:], Alu.add)
            GP.tensor_tensor(rn[:], beq[:], w[:, 0:1], Alu.subtract)
            GP.tensor_tensor(dg[:], d[:, :10], d[:, 10:20], Alu.add)

            # tableau assembly
            V.tensor_tensor(Tv[:, :, 0:10], mmC_S, QFv, Alu.add)
            SC.copy(Tv[:, :, 10:12], E0Pv)
            V.tensor_tensor(Tdiag, Tdiag, dg[:], Alu.add)
            V.tensor_tensor(t_mbg[:], mmU[:, 10:20], t_zhp[:], Alu.subtract)
            V.tensor_tensor(Tv[:, :, 10:11],
                            t_mbg[:].rearrange("p (r c) -> p r c", c=1),
                            mmU[:, 0:10].rearrange("p (r c) -> p r c", c=1),
                            Alu.subtract)
            V.tensor_tensor(T[:, 10:11], T[:, 10:11], nuneg[:], Alu.add)

            # batched 10x10 Gauss-Jordan, 2 rhs (rt | e0)
            for k in range(10):
                V.reciprocal(recips[:, k:k + 1], T[:, 13 * k:13 * k + 1])
                mcols = 11 - k
                pv = prod[:, :10 * mcols].rearrange("p (r c) -> p r c", c=mcols)
                V.scalar_tensor_tensor(
                    pv,
                    Tv[:, :, k:k + 1].broadcast_to((P, 10, mcols)),
                    recips[:, k:k + 1],
                    Tv[:, k:k + 1, k + 1:].broadcast_to((P, 10, mcols)),
                    Alu.mult, Alu.mult)
                if k < 9:
                    V.tensor_tensor(Tv[:, k + 1:, k + 1:],
                                    Tv[:, k + 1:, k + 1:], pv[:, k + 1:, :],
                                    Alu.subtract)
                if k > 0:
                    GP.tensor_tensor(Tv[:, :k, k + 1:],
                                     Tv[:, :k, k + 1:], pv[:, :k, :],
                                     Alu.subtract)
            V.tensor_tensor(Xv, Tv[:, :, 10:12],
                            recv.broadcast_to((P, 10, 2)), Alu.mult)

            # dnu (negated), dz
            V.tensor_scalar(den[:], X[:, 1:2], REG, None, Alu.add)
            V.reciprocal(rden[:], den[:])
            V.tensor_tensor(num2[:], rn[:], X[:, 0:1], Alu.subtract)
            V.tensor_tensor(dnu_neg[:], num2[:], rden[:], Alu.mult)
            V.scalar_tensor_tensor(
                dw[:, :10].rearrange("p (r c) -> p r c", c=1),
                Xv[:, :, 1:2], dnu_neg[:], Xv[:, :, 0:1], Alu.mult, Alu.add)

            # MM3: Bhdz
            TE.transpose(tps[:NZ, :], dw[:, :NZ], ID[:])
            V.tensor_copy(tT[:NZ, :], tps[:NZ, :])
            TE.matmul(mmA[:, :40], tT[:NZ, :], R1[:NZ, :40])

            # SD parts that need only dz (overlap MM3)
            V.tensor_tensor(SDD[:, :10], dw[:, :10], rp[:, :10], Alu.add)
            V.tensor_tensor(SDD[:, 10:20], rp[:, 10:20], dw[:, :10],
                            Alu.subtract)

            # de = (rw_e + d2*[Bhdz;-Bhdz]) / me
            V.tensor_tensor(
                t80ab[:].rearrange("p (r c) -> p r c", c=40),
                d[:, 20:100].rearrange("p (r c) -> p r c", c=40),
                mmA[:, :40].unsqueeze(1).broadcast_to((P, 2, 40)),
                Alu.mult)
            V.tensor_tensor(de80[:, :40], rw_e[:, :40], t80ab[:, :40], Alu.add)
            V.tensor_tensor(de80[:, 40:], rw_e[:, 40:], t80ab[:, 40:],
                            Alu.subtract)
            V.tensor_tensor(dw[:, 10:90], de80[:], rme[:], Alu.mult)

            # SD = -(ds) = G dw + r_prim (rest)
            V.tensor_tensor(u80[:], rp[:, 20:100], dw[:, 10:90], Alu.subtract)
            V.tensor_tensor(SDD[:, 20:60], u80[:, :40], mmA[:, :40], Alu.add)
            V.tensor_tensor(SDD[:, 60:100], u80[:, 40:], mmA[:, :40],
                            Alu.subtract)
            V.tensor_tensor(SDD[:, 100:180], rp[:, 100:180], dw[:, 10:90],
                            Alu.subtract)

            # -dlam = (lam - mus) - d*SD
            V.tensor_tensor(t180[:], d[:], SD_, Alu.mult)
            V.tensor_tensor(dlN_, v1[:], t180[:], Alu.subtract)

            # alpha = 0.99*min(1, min([s|lam] / max(SDD, eps)))
            V.tensor_scalar(sm[:], SDD[:], 1e-12, None, Alu.max)
            V.reciprocal(rm[:], sm[:])
            V.tensor_tensor(ratio[:], sl[:], rm[:], Alu.mult)
            V.tensor_reduce(a1t[:], ratio[:], mybir.AxisListType.X, Alu.min)
            V.tensor_scalar(alpha[:], a1t[:], 1.0, 0.99, Alu.min, Alu.mult)
            V.tensor_scalar(alphan[:], alpha[:], -1.0, None, Alu.mult)

            # updates
            V.scalar_tensor_tensor(w[:], dw[:], alpha[:], w[:],
                                   Alu.mult, Alu.add)
            if it < iters - 1:
                V.scalar_tensor_tensor(sl[:], SDD[:], alphan[:], sl[:],
                                       Alu.mult, Alu.add)
                V.scalar_tensor_tensor(nuneg[:], dnu_neg[:], alpha[:],
                                       nuneg[:], Alu.mult, Alu.add)

        # ------------------------------------------------ final cost
        TE.transpose(tps[:NZ, :], w[:, :NZ], ID[:])
        SC.copy(tT[:NZ, :], tps[:NZ, :])
        TE.matmul(mmA[:, :50], tT[:NZ, :], R1[:NZ, :])
        V.scalar_tensor_tensor(t_zhp[:], mmA[:, 40:50], 0.5, p[:],
                               Alu.mult, Alu.add)        # u@Q_hat + p
        V.scalar_tensor_tensor(junk10[:], t_zhp[:], 1.0, w[:, :10],
                               Alu.mult, Alu.mult, accum_out=bt_[:])
        V.scalar_tensor_tensor(junk80[:], w[:, 10:], 1.0, w[:, 10:],
                               Alu.mult, Alu.mult, accum_out=ct_[:])
        V.tensor_tensor(costsb[:], bt_[:], ct_[:], Alu.add)
        V.tensor_tensor(costsb[:], costsb[:], bc[:], Alu.add)
        nc.sync.dma_start(cost_out, costsb[:])
        nc.sync.dma_start(u0_out, w[:, 0:1])

    nc.compile()
    return nc


# ------------------------------------------------------------------- driver
_CACHE = {}


def _get_program(iters=ITERS):
    if iters not in _CACHE:
        _CACHE[iters] = build_program(iters)
    return _CACHE[iters]


def make_in_maps(x, u0, Q_sqrt, R_sqrt, A_dyn, B_dyn):
    fnp = np.float32
    consts = _host_constants(np.asarray(Q_sqrt, fnp), np.asarray(R_sqrt, fnp),
                             np.asarray(A_dyn, fnp), np.asarray(B_dyn, fnp))
    us = np.ascontiguousarray(np.asarray(u0, fnp).reshape(N_CORES, BPC, 1))
    shared = {
        "r1": consts["R1"], "p40": consts["P40"],
        "rug": consts["RBug"], "qf": consts["QF"], "e0p": consts["E0P"],
        "ident": consts["ID"],
    }
    xf = np.asarray(x, fnp)
    A_x0 = (xf @ consts["C1"]).astype(fnp)                      # (B,40)
    h2f = np.concatenate([4.0 - A_x0, 4.0 + A_x0], axis=1).astype(fnp)
    pf = (xf @ consts["C2"]).astype(fnp)                        # (B,10)
    bcf = (np.sum((xf @ consts["W4"]) * xf, axis=1)
           + np.sum((xf @ consts["Q"]) * xf, axis=1)).astype(fnp)[:, None]
    h2s = h2f.reshape(N_CORES, BPC, NE)
    ps_ = pf.reshape(N_CORES, BPC, NZ)
    bcs = bcf.reshape(N_CORES, BPC, 1)
    return [dict(shared, h2in=h2s[c], pin=ps_[c], bcin=bcs[c], u0in=us[c])
            for c in range(N_CORES)]


def kernel(x, u0, Q_sqrt, R_sqrt, A_dyn, B_dyn):
    nc = _get_program()
    in_maps = make_in_maps(x, u0, Q_sqrt, R_sqrt, A_dyn, B_dyn)
    res = run_bass_kernel_spmd(nc, in_maps, list(range(N_CORES)))
    cost = np.concatenate([res.results[c]["cost"] for c in range(N_CORES)],
                          axis=0).astype(np.float32)
    u0o = np.concatenate([res.results[c]["u0o"] for c in range(N_CORES)],
                         axis=0).reshape(B).astype(np.float32)
    return cost, u0o


# revision 16
# speedup vs baseline: 1.1958x; 1.1404x over previous
"""Trainium2 Bass kernel for the batched convex-MPC QP (nn_Cvx_Nets).

Strategy (pure data parallel, 8 cores x 128 samples):
  * Host precomputes the tiny shared MPC matrices (A_hat, B_hat, Q_hat, ...)
    from A_dyn/B_dyn/Q_sqrt/R_sqrt in numpy (O(1) work).
  * Each core solves 128 independent QPs with a fixed-iteration primal-dual
    interior point method, one sample per SBUF partition, all per-sample
    vectors along the free dimension.
  * The 91x91 KKT solve of the reference collapses: w=[z;e] with the e-block
    of the Schur matrix diagonal and a rank-1 equality constraint, so each
    IPM step needs only a per-sample 10x10 solve, done as batched
    Gauss-Jordan elimination with broadcast access patterns on the vector
    engine.  The few per-sample matvecs with shared small matrices map to
    TensorE matmuls (samples on the output partition dim).
  * The IPM is fully converged long before the reference's 20 iterations;
    we run ITERS (validated: rel err ~2e-5 at 10 iters vs the reference).
  * s,lam share one [128,360] tile (sl) and -ds,-dlam share another (SDD) so
    the step-length search and the state update are single wide DVE ops.
    The independent cc-chain runs on GpSimd+Scalar engines in parallel with
    the DVE residual chain.
"""

import numpy as np
from contextlib import ExitStack

import concourse.bacc as bacc
import concourse.tile as tile
import concourse.mybir as mybir
from concourse.bass_utils import run_bass_kernel_spmd

f32 = mybir.dt.float32
Alu = mybir.AluOpType
Act = mybir.ActivationFunctionType

N_CORES = 8
B = 1024
BPC = B // N_CORES          # 128 samples per core = partition dim
NZ, NE, NW = 10, 80, 90
MI = 180
SIGMA, REG = 0.1, 1e-8
N = 10                      # horizon
NI = 4                      # state dim
ITERS = 8


# ---------------------------------------------------------------- host prep
def _host_constants(Q_sqrt, R_sqrt, A, B_dyn):
    fnp = np.float32
    Apows = [A]
    for _ in range(1, N):
        Apows.append((Apows[-1] @ A).astype(fnp))
    A_hat = np.concatenate(Apows, axis=0)                       # (40,4)
    T = [B_dyn]
    for _ in range(1, N):
        T.append((A @ T[-1]).astype(fnp))
    T = np.stack(T)                                             # (N,4,1)
    i = np.arange(N)
    diff = i[:, None] - i[None, :]
    blocks = np.where((diff >= 0)[:, :, None, None],
                      T[np.clip(diff, 0, N - 1)], 0.0).astype(fnp)
    Bh = blocks.transpose(0, 2, 1, 3).reshape(N * NI, N).astype(fnp)  # (40,10)
    Q = (Q_sqrt @ Q_sqrt.T).astype(fnp)
    R = (R_sqrt @ R_sqrt.T).astype(fnp)
    Q_diag = np.kron(np.eye(N, dtype=fnp), Q)                   # (40,40)
    R_diag = np.kron(np.eye(N, dtype=fnp), R)                   # (10,10)
    Q_hat = (Bh.T @ (Q_diag @ Bh) + R_diag).astype(fnp)         # (10,10)
    Qhat2 = (2.0 * Q_hat).astype(fnp)

    C1 = A_hat.T.astype(fnp)                                    # (4,40)
    C2 = (2.0 * (A_hat.T @ (Q_diag @ Bh))).astype(fnp)          # (4,10)
    W4 = (A_hat.T @ Q_diag @ A_hat).astype(fnp)                 # (4,4)

    R1 = np.concatenate([Bh.T, Qhat2], axis=1).astype(fnp)      # (10,50)

    P40 = np.einsum('ki,kj->kij', Bh, Bh).reshape(40, 100).astype(fnp)
    RBug = np.zeros((80, 20), fnp)                              # [u_vec|gdiff]@.
    RBug[0:40, 0:10] = Bh
    RBug[40:80, 10:20] = Bh

    QFrow = (Qhat2 + REG * np.eye(NZ, dtype=fnp)).reshape(-1).astype(fnp)
    QF = np.ascontiguousarray(np.broadcast_to(QFrow, (BPC, 100))).astype(fnp)

    E0P = np.zeros((BPC, 20), fnp)
    E0P[:, 1] = 1.0                                             # e0 rhs col, row 0

    ID = np.eye(BPC, dtype=fnp)
    return dict(R1=R1, P40=P40, RBug=RBug, QF=QF, E0P=E0P, ID=ID,
                C1=C1, C2=C2, W4=W4, Q=Q)


# ------------------------------------------------------------- device build
def build_program(iters=ITERS):
    nc = bacc.Bacc("TRN2", target_bir_lowering=False, debug=False)
    P = BPC

    h2in = nc.dram_tensor("h2in", [P, NE], f32, kind="ExternalInput").ap()
    pin = nc.dram_tensor("pin", [P, NZ], f32, kind="ExternalInput").ap()
    bcin = nc.dram_tensor("bcin", [P, 1], f32, kind="ExternalInput").ap()
    u0in = nc.dram_tensor("u0in", [P, 1], f32, kind="ExternalInput").ap()
    r1d = nc.dram_tensor("r1", [NZ, 50], f32, kind="ExternalInput").ap()
    p40d = nc.dram_tensor("p40", [40, 100], f32, kind="ExternalInput").ap()
    rugd = nc.dram_tensor("rug", [80, 20], f32, kind="ExternalInput").ap()
    qfd = nc.dram_tensor("qf", [P, 100], f32, kind="ExternalInput").ap()
    e0d = nc.dram_tensor("e0p", [P, 20], f32, kind="ExternalInput").ap()
    idd = nc.dram_tensor("ident", [P, P], f32, kind="ExternalInput").ap()
    cost_out = nc.dram_tensor("cost", [P, 1], f32, kind="ExternalOutput").ap()
    u0_out = nc.dram_tensor("u0o", [P, 1], f32, kind="ExternalOutput").ap()

    with tile.TileContext(nc) as tc, ExitStack() as ctx:
        sb = ctx.enter_context(tc.tile_pool(name="sb", bufs=1))
        ps = ctx.enter_context(tc.tile_pool(name="ps", bufs=1, space="PSUM"))

        def st(shape, tag):
            return sb.tile(list(shape), f32, tag=tag, name=tag)

        # constants
        R1 = st((P, 50), "R1")
        P40 = st((P, 100), "P40"); RUG = st((P, 20), "RUG")
        QF = st((P, 100), "QF"); E0P = st((P, 20), "E0P"); ID = st((P, P), "ID")
        # state
        beq = st((P, 1), "beq")
        w = st((P, NW), "w")
        sl = st((P, 2 * MI), "sl")          # [s | lam]
        nuneg = st((P, 1), "nuneg")
        p = st((P, NZ), "p"); h2 = st((P, NE), "h2"); bc = st((P, 1), "bc")
        # per-iteration tensors
        rp = st((P, MI), "rp"); rs = st((P, MI), "rs"); d = st((P, MI), "d")
        mus = st((P, MI), "mus"); lt = st((P, MI), "lt")
        t180 = st((P, MI), "t180"); v1 = st((P, MI), "v1")
        me1 = st((P, NE), "me1"); me = st((P, NE), "me"); rme = st((P, NE), "rme")
        t_cc1 = st((P, NE), "t_cc1"); t_cc2 = st((P, NE), "t_cc2")
        cct = st((P, 40), "cct")            # cc40 (gpsimd-owned)
        t_rpa = st((P, NE), "t_rpa"); t_rpb = st((P, NE), "t_rpb")
        t_lt23 = st((P, NE), "t_lt23"); rw_e = st((P, NE), "rw_e")
        t_ga = st((P, NE), "t_ga"); t_gb = st((P, NE), "t_gb")
        vtcat = st((P, 80), "vtcat")        # [u_vec | gdiff]
        t_lt1d = st((P, NZ), "t_lt1d"); t_zhp = st((P, NZ), "t_zhp")
        t_mbg = st((P, NZ), "t_mbg"); dg = st((P, NZ), "dg")
        rn = st((P, 1), "rn")
        T = st((P, 120), "T"); recips = st((P, NZ), "recips")
        prod = st((P, 112), "prod"); X = st((P, 20), "X")
        den = st((P, 1), "den"); rden = st((P, 1), "rden")
        num2 = st((P, 1), "num2"); dnu_neg = st((P, 1), "dnu_neg")
        dw = st((P, NW), "dw"); de80 = st((P, NE), "de80")
        t80ab = st((P, NE), "t80ab"); u80 = st((P, NE), "u80")
        SDD = st((P, 2 * MI), "SDD")        # [-ds | -dlam]
        sm = st((P, 2 * MI), "sm"); rm = st((P, 2 * MI), "rm")
        ratio = st((P, 2 * MI), "ratio")
        junkA = st((P, MI), "junkA"); junk10 = st((P, NZ), "junk10")
        junk80 = st((P, NE), "junk80")
        slam = st((P, 1), "slam")
        a1t = st((P, 1), "a1t")
        alpha = st((P, 1), "alpha"); alphan = st((P, 1), "alphan")
        bt_ = st((P, 1), "bt_"); ct_ = st((P, 1), "ct_")
        costsb = st((P, 1), "costsb")
        tT = st((P, P), "tT"); tTc = st((P, P), "tTc")

        tps = ps.tile([P, P], f32, tag="tps", name="tps")
        tpsC = ps.tile([P, P], f32, tag="tpsC", name="tpsC")
        mmA = ps.tile([P, 64], f32, tag="mmA", name="mmA")
        mmU = ps.tile([P, 32], f32, tag="mmU", name="mmU")
        mmC = ps.tile([P, 128], f32, tag="mmC", name="mmC")

        V = nc.vector
        SC = nc.scalar
        TE = nc.tensor
        GP = nc.gpsimd

        # views
        s_ = sl[:, 0:MI]
        lam_ = sl[:, MI:2 * MI]
        SD_ = SDD[:, 0:MI]
        dlN_ = SDD[:, MI:2 * MI]
        Tv = T[:].rearrange("p (r c) -> p r c", c=12)
        Tdiag = T[:, 0:120:13]
        Xv = X[:].rearrange("p (r c) -> p r c", c=2)
        E0Pv = E0P[:].rearrange("p (r c) -> p r c", c=2)
        mmC_S = mmC[:, 0:100].rearrange("p (r c) -> p r c", c=10)
        QFv = QF[:].rearrange("p (r c) -> p r c", c=10)
        recv = recips[:].rearrange("p (r c) -> p r c", c=1)

        # ------------------------------------------------ load + init
        nc.sync.dma_start(h2[:], h2in)
        nc.sync.dma_start(p[:], pin)
        nc.sync.dma_start(bc[:], bcin)
        nc.sync.dma_start(beq[:], u0in)
        nc.sync.dma_start(R1[:NZ, :], r1d)
        nc.sync.dma_start(P40[:40, :], p40d)
        nc.sync.dma_start(RUG[:80, :], rugd)
        nc.sync.dma_start(QF[:], qfd)
        nc.sync.dma_start(E0P[:], e0d)
        nc.sync.dma_start(ID[:], idd)

        GP.memset(sl[:, 0:20], 1.0)
        V.tensor_scalar(sl[:, 20:100], h2[:], 1.0, None, Alu.max)
        GP.memset(sl[:, 100:180], 1.0)
        GP.memset(lam_, 1.0)
        GP.memset(w[:], 0.0)
        GP.memset(nuneg[:], 0.0)

        # ------------------------------------------------ IPM iterations
        for it in range(iters):
            # z transpose + MM1: Bhz | z@2Q_hat
            TE.transpose(tps[:NZ, :], w[:, :NZ], ID[:])
            SC.copy(tT[:NZ, :], tps[:NZ, :])
            TE.matmul(mmA[:, :50], tT[:NZ, :], R1[:NZ, :])

            # mu, rs, d  (independent of MM1)
            V.scalar_tensor_tensor(junkA[:], s_, 1.0, lam_,
                                   Alu.mult, Alu.mult, accum_out=slam[:])
            V.reciprocal(rs[:], s_)
            V.tensor_tensor(d[:], lam_, rs[:], Alu.mult)
            V.tensor_scalar(mus[:], rs[:], slam[:], SIGMA / MI,
                            Alu.mult, Alu.mult)
            # cc-chain: cc40 = d2 - d2*(d2/me) (+halves); me1/t_cc1 on GpSimd
            GP.tensor_tensor(me1[:], d[:, 20:100], d[:, 100:180], Alu.add)
            SC.activation(me[:], me1[:], Act.Copy, bias=2.0)
            V.reciprocal(rme[:], me[:])
            GP.tensor_tensor(t_cc1[:], d[:, 20:100], rme[:], Alu.mult)
            V.tensor_tensor(t_cc2[:], t_cc1[:], d[:, 20:100], Alu.mult)
            V.tensor_tensor(t_cc2[:], d[:, 20:100], t_cc2[:], Alu.subtract)
            V.tensor_tensor(cct[:], t_cc2[:, :40], t_cc2[:, 40:], Alu.add)
            # cc transpose + MM-cc: Sg = cc40 @ P40   (early, overlaps)
            TE.transpose(tpsC[:40, :], cct[:], ID[:])
            SC.copy(tTc[:40, :], tpsC[:40, :])
            TE.matmul(mmC[:, :100], tTc[:40, :], P40[:40, :])

            # r_prim (needs Bhz)
            V.scalar_tensor_tensor(rp[:, :10], sl[:, :10], -0.5, w[:, :10],
                                   Alu.add, Alu.add)
            V.scalar_tensor_tensor(rp[:, 10:20], sl[:, 10:20], -0.5, w[:, :10],
                                   Alu.add, Alu.subtract)
            GP.tensor_tensor(t_rpa[:], sl[:, 20:100], h2[:], Alu.subtract)
            GP.tensor_tensor(t_rpb[:], t_rpa[:], w[:, 10:90], Alu.subtract)
            V.tensor_tensor(rp[:, 20:60], t_rpb[:, :40], mmA[:, :40], Alu.add)
            V.tensor_tensor(rp[:, 60:100], t_rpb[:, 40:], mmA[:, :40],
                            Alu.subtract)
            V.tensor_tensor(rp[:, 100:180], sl[:, 100:180], w[:, 10:90],
                            Alu.subtract)

            # lt = lam + tmp = d*rp + mus;  v1 = lam - mus (for dlam, gpsimd)
            V.tensor_tensor(t180[:], d[:], rp[:], Alu.mult)
            V.tensor_tensor(lt[:], t180[:], mus[:], Alu.add)
            GP.tensor_tensor(v1[:], lam_, mus[:], Alu.subtract)

            # gdiff chain (feeds MM-ug): rw_e, g
            V.tensor_tensor(t_lt23[:], lt[:, 20:100], lt[:, 100:180], Alu.add)
            V.scalar_tensor_tensor(rw_e[:], w[:, 10:90], -2.0, t_lt23[:],
                                   Alu.mult, Alu.add)
            V.tensor_tensor(t_gb[:], t_cc1[:], rw_e[:], Alu.mult)
            V.tensor_tensor(vtcat[:, 40:80], t_gb[:, :40], t_gb[:, 40:],
                            Alu.subtract)
            V.tensor_tensor(vtcat[:, :40], lt[:, 20:60], lt[:, 60:100],
                            Alu.subtract)

            # MM-ug: [u_vec|gdiff] @ blockdiag(Bh,Bh) -> MB_u | MB_g
            TE.transpose(tps[:80, :], vtcat[:], ID[:])
            V.tensor_copy(tT[:80, :], tps[:80, :])
            TE.matmul(mmU[:, :20], tT[:80, :], RUG[:80, :])

            # rw_z pieces, rn (overlap MM-ug)
            GP.tensor_tensor(t_lt1d[:], lt[:, :10], lt[:, 10:20], Alu.subtract)
            V.tensor_tensor(t_zhp[:], mmA[:, 40:50], p[:], Alu.add)
            V.tensor_tensor(t_zhp[:], t_zhp[:], t_lt1d[:], Alu.add)
            GP.tensor_tensor(rn[:], beq[:], w[:, 0:1], Alu.subtract)
            GP.tensor_tensor(dg[:], d[:, :10], d[:, 10:20], Alu.add)

            # tableau assembly
            V.tensor_tensor(Tv[:, :, 0:10], mmC_S, QFv, Alu.add)
            SC.copy(Tv[:, :, 10:12], E0Pv)
            V.tensor_tensor(Tdiag, Tdiag, dg[:], Alu.add)
            V.tensor_tensor(t_mbg[:], mmU[:, 10:20], t_zhp[:], Alu.subtract)
            V.tensor_tensor(Tv[:, :, 10:11],
                            t_mbg[:].rearrange("p (r c) -> p r c", c=1),
                            mmU[:, 0:10].rearrange("p (r c) -> p r c", c=1),
                            Alu.subtract)
            V.tensor_tensor(T[:, 10:11], T[:, 10:11], nuneg[:], Alu.add)

            # batched 10x10 Gauss-Jordan, 2 rhs (rt | e0)
            for k in range(10):
                V.reciprocal(recips[:, k:k + 1], T[:, 13 * k:13 * k + 1])
                mcols = 11 - k
                pv = prod[:, :10 * mcols].rearrange("p (r c) -> p r c", c=mcols)
                V.scalar_tensor_tensor(
                    pv,
                    Tv[:, :, k:k + 1].broadcast_to((P, 10, mcols)),
                    recips[:, k:k + 1],
                    Tv[:, k:k + 1, k + 1:].broadcast_to((P, 10, mcols)),
                    Alu.mult, Alu.mult)
                if k < 9:
                    V.tensor_tensor(Tv[:, k + 1:, k + 1:],
                                    Tv[:, k + 1:, k + 1:], pv[:, k + 1:, :],
                                    Alu.subtract)
                if k > 0:
                    V.tensor_tensor(Tv[:, :k, k + 1:],
                                    Tv[:, :k, k + 1:], pv[:, :k, :],
                                    Alu.subtract)
            V.tensor_tensor(Xv, Tv[:, :, 10:12],
                            recv.broadcast_to((P, 10, 2)), Alu.mult)

            # dnu (negated), dz
            V.tensor_scalar(den[:], X[:, 1:2], REG, None, Alu.add)
            V.reciprocal(rden[:], den[:])
            V.tensor_tensor(num2[:], rn[:], X[:, 0:1], Alu.subtract)
            V.tensor_tensor(dnu_neg[:], num2[:], rden[:], Alu.mult)
            V.scalar_tensor_tensor(
                dw[:, :10].rearrange("p (r c) -> p r c", c=1),
                Xv[:, :, 1:2], dnu_neg[:], Xv[:, :, 0:1], Alu.mult, Alu.add)

            # MM3: Bhdz
            TE.transpose(tps[:NZ, :], dw[:, :NZ], ID[:])
            V.tensor_copy(tT[:NZ, :], tps[:NZ, :])
            TE.matmul(mmA[:, :40], tT[:NZ, :], R1[:NZ, :40])

            # SD parts that need only dz (overlap MM3)
            V.tensor_tensor(SDD[:, :10], dw[:, :10], rp[:, :10], Alu.add)
            V.tensor_tensor(SDD[:, 10:20], rp[:, 10:20], dw[:, :10],
                            Alu.subtract)

            # de = (rw_e + d2*[Bhdz;-Bhdz]) / me
            V.tensor_tensor(
                t80ab[:].rearrange("p (r c) -> p r c", c=40),
                d[:, 20:100].rearrange("p (r c) -> p r c", c=40),
                mmA[:, :40].unsqueeze(1).broadcast_to((P, 2, 40)),
                Alu.mult)
            V.tensor_tensor(de80[:, :40], rw_e[:, :40], t80ab[:, :40], Alu.add)
            V.tensor_tensor(de80[:, 40:], rw_e[:, 40:], t80ab[:, 40:],
                            Alu.subtract)
            V.tensor_tensor(dw[:, 10:90], de80[:], rme[:], Alu.mult)

            # SD = -(ds) = G dw + r_prim (rest)
            V.tensor_tensor(u80[:], rp[:, 20:100], dw[:, 10:90], Alu.subtract)
            V.tensor_tensor(SDD[:, 20:60], u80[:, :40], mmA[:, :40], Alu.add)
            V.tensor_tensor(SDD[:, 60:100], u80[:, 40:], mmA[:, :40],
                            Alu.subtract)
            V.tensor_tensor(SDD[:, 100:180], rp[:, 100:180], dw[:, 10:90],
                            Alu.subtract)

            # -dlam = (lam - mus) - d*SD
            V.tensor_tensor(t180[:], d[:], SD_, Alu.mult)
            V.tensor_tensor(dlN_, v1[:], t180[:], Alu.subtract)

            # alpha = 0.99*min(1, min([s|lam] / max(SDD, eps)))
            V.tensor_scalar(sm[:], SDD[:], 1e-12, None, Alu.max)
            V.reciprocal(rm[:], sm[:])
            V.tensor_tensor(ratio[:], sl[:], rm[:], Alu.mult)
            V.tensor_reduce(a1t[:], ratio[:], mybir.AxisListType.X, Alu.min)
            V.tensor_scalar(alpha[:], a1t[:], 1.0, 0.99, Alu.min, Alu.mult)
            V.tensor_scalar(alphan[:], alpha[:], -1.0, None, Alu.mult)

            # updates
            V.scalar_tensor_tensor(w[:], dw[:], alpha[:], w[:],
                                   Alu.mult, Alu.add)
            if it < iters - 1:
                V.scalar_tensor_tensor(sl[:], SDD[:], alphan[:], sl[:],
                                       Alu.mult, Alu.add)
                V.scalar_tensor_tensor(nuneg[:], dnu_neg[:], alpha[:],
                                       nuneg[:], Alu.mult, Alu.add)

        # ------------------------------------------------ final cost
        TE.transpose(tps[:NZ, :], w[:, :NZ], ID[:])
        SC.copy(tT[:NZ, :], tps[:NZ, :])
        TE.matmul(mmA[:, :50], tT[:NZ, :], R1[:NZ, :])
        V.scalar_tensor_tensor(t_zhp[:], mmA[:, 40:50], 0.5, p[:],
                               Alu.mult, Alu.add)        # u@Q_hat + p
        V.scalar_tensor_tensor(junk10[:], t_zhp[:], 1.0, w[:, :10],
                               Alu.mult, Alu.mult, accum_out=bt_[:])
        V.scalar_tensor_tensor(junk80[:], w[:, 10:], 1.0, w[:, 10:],
                               Alu.mult, Alu.mult, accum_out=ct_[:])
        V.tensor_tensor(costsb[:], bt_[:], ct_[:], Alu.add)
        V.tensor_tensor(costsb[:], costsb[:], bc[:], Alu.add)
        nc.sync.dma_start(cost_out, costsb[:])
        nc.sync.dma_start(u0_out, w[:, 0:1])

    nc.compile()
    return nc


# ------------------------------------------------------------------- driver
_CACHE = {}


def _get_program(iters=ITERS):
    if iters not in _CACHE:
        _CACHE[iters] = build_program(iters)
    return _CACHE[iters]


def make_in_maps(x, u0, Q_sqrt, R_sqrt, A_dyn, B_dyn):
    fnp = np.float32
    consts = _host_constants(np.asarray(Q_sqrt, fnp), np.asarray(R_sqrt, fnp),
                             np.asarray(A_dyn, fnp), np.asarray(B_dyn, fnp))
    us = np.ascontiguousarray(np.asarray(u0, fnp).reshape(N_CORES, BPC, 1))
    shared = {
        "r1": consts["R1"], "p40": consts["P40"],
        "rug": consts["RBug"], "qf": consts["QF"], "e0p": consts["E0P"],
        "ident": consts["ID"],
    }
    xf = np.asarray(x, fnp)
    A_x0 = (xf @ consts["C1"]).astype(fnp)                      # (B,40)
    h2f = np.concatenate([4.0 - A_x0, 4.0 + A_x0], axis=1).astype(fnp)
    pf = (xf @ consts["C2"]).astype(fnp)                        # (B,10)
    bcf = (np.sum((xf @ consts["W4"]) * xf, axis=1)
           + np.sum((xf @ consts["Q"]) * xf, axis=1)).astype(fnp)[:, None]
    h2s = h2f.reshape(N_CORES, BPC, NE)
    ps_ = pf.reshape(N_CORES, BPC, NZ)
    bcs = bcf.reshape(N_CORES, BPC, 1)
    return [dict(shared, h2in=h2s[c], pin=ps_[c], bcin=bcs[c], u0in=us[c])
            for c in range(N_CORES)]


def kernel(x, u0, Q_sqrt, R_sqrt, A_dyn, B_dyn):
    nc = _get_program()
    in_maps = make_in_maps(x, u0, Q_sqrt, R_sqrt, A_dyn, B_dyn)
    res = run_bass_kernel_spmd(nc, in_maps, list(range(N_CORES)))
    cost = np.concatenate([res.results[c]["cost"] for c in range(N_CORES)],
                          axis=0).astype(np.float32)
    u0o = np.concatenate([res.results[c]["u0o"] for c in range(N_CORES)],
                         axis=0).reshape(B).astype(np.float32)
    return cost, u0o
